# revision 34
# baseline (speedup 1.0000x reference)
"""Trainium2 Bass kernel for 4M per-element tiny MLPs (1->8->1, H=8).

    y[i] = W2[i] @ relu(W1[i] * x[i] + b1[i]) + b2[i]

Memory-bound; data-parallel over 8 NeuronCores (500k nets/core), no
communication.

Design (evolution of the fp16 j-major slab baseline, 104.8 us):
  * Mid tiles store W1 as fp8 e3m4 (TRN float8e3: 4 mantissa bits; W1 in
    (-1,1) so no overflow; end-to-end rel_l2 7.5e-3 vs the 2e-2 gate).
    Slab 52 -> 44 B/net; the fp8->fp16 upconvert runs on the ACT engine,
    which has slack (relu+upconvert ~6.8 us < DVE ~8.4 us per 480-tile).
    DVE op mix is untouched (its operands stay fp16 at 2x_1p).
  * Edge tiles (first 2, last 2) keep W1 in fp16 and run relu on DVE
    (tensor_scalar_max @ 4x): their za/zb/relu/tail chains touch no other
    engine, so pipeline fill and drain carry no cross-engine sem
    latency. Costs ~1.5 us of DVE across the 739 edge nets/partition.
  * ALL slab DMAs ride the sync queue, issued 2 tiles ahead (except
    slab 1, issued on the scalar queue before any ACTIVATE so the first
    two transfers overlap). v3's lesson: a DMA_DIRECT2D queued on the
    scalar engine right before its consuming ACTIVATE head-of-line
    blocks the ACT queue for the whole transfer. With fp8 bytes + big
    tiles one HW DGE queue sustains ~430 GB/s and fits in the period.
  * ACT queue order per tile: upconv(t+1) then relu(t) - upconv rides
    one tile ahead so relu's zb-wait never delays it.
  * y-writes go via the GPSIMD software-DGE queue (keeps them off both
    the HW DGE queue and the ACT queue), except the last two tiles',
    which ride the by-then-idle scalar HW queue so the final drain
    barrier does not wait on SWDGE completion lag. Output is fp16.
  * Slab = one uint8 DMA per tile; typed views via bitcast:
    fp8 tile: [W1 f8 j-major 8fi | b1 f16 j-major 16fi | W2 f16 j-major
    16fi | x f16 2fi | b2 f16 2fi] B/partition (44fi); fp16 tile: same
    with W1 f16 16fi first (52fi).

Measured (run-to-run spread +-3-6us, HBM shared with co-tenants):
v1 all-fp16 104.8us (DMA-period-bound: 81.7us DMA active > 74.9 DVE);
fp8+ACT-upconvert with naive queueing 117-119us (ACT queue overloaded by
DMA_DIRECT2D + upconverts, and up(t+1) queued behind relu(t) serialized
the pipe); single-sync-queue DMA + 2-ahead prefetch + 1-ahead upconvert
101.4us; +fp16/DVE-relu edge tiles 101.2us; +last-2 y-outs on the idle
scalar HW queue 101.06us (best; DVE-bound: ~75.4us busy, DMA active 61,
ACT 50; remaining span = ~5.5us NEFF preamble + ~3us first-transfer DGE
startup latency + ~9us ramp fill + ~8.5us drain barrier). Dead ends
measured: upconvert hoisted 2 ahead with DMA 3 ahead (+4us); y-outs via
sync queue (+4us; sync still holds slab completion sems); finer 13-tile
taper (+4.5us: per-tile op+sem overhead beats ramp savings); 3-queue
prelude with slab1 on gpsimd SWDGE (+7.8us: SWDGE descriptor gen too
slow for MB transfers); PE-engine tail reduction (ones-matmul
j-on-partitions, verified exact in mini_pe.py) - killed by "DMA cannot
read PSUM": the PSUM->SBUF copy at 1 elem/net on 16 partitions costs
more than the DVE tree it removes. Structural wall: DVE 16 cyc/net
(tensor_tensor 2x floor; custom DVE ops are 1x, GPSIMD ~4x slower +
steals the DVE SBUF port).
"""

import numpy as np
import ml_dtypes
from contextlib import ExitStack

import concourse.bacc as bacc
import concourse.mybir as mybir
import concourse.tile as tile
from concourse.bass_utils import run_bass_kernel_spmd

F16 = mybir.dt.float16
F8 = mybir.dt.float8e3
U8 = mybir.dt.uint8
AF = mybir.ActivationFunctionType
OP = mybir.AluOpType
E3M4 = ml_dtypes.float8_e3m4

N = 4_000_000
H = 8
N_CORES = 8
R = N // N_CORES            # 500,000 nets per core
FP = 3907                   # nets per partition (padded): 128*3907 = 500,096
R_PAD = 128 * FP
FIS = [64, 224, 288] + [480] * 6 + [252, 199]
FP8T = [False, False, True, True, True, True, True, True, True, False, False]


def slab_b(ti):
    return 44 if FP8T[ti] else 52


def build_nc(fis):
    nt = len(fis)
    tile_bytes = [128 * fis[t] * slab_b(t) for t in range(nt)]
    offs = [0]
    for tb in tile_bytes:
        offs.append(offs[-1] + tb)
    rbs = [0]
    for fi in fis:
        rbs.append(rbs[-1] + 128 * fi)

    nc = bacc.Bacc("TRN2", target_bir_lowering=False, debug=False)

    slab = nc.dram_tensor("slab", [offs[-1]], U8, kind="ExternalInput")
    ys = nc.dram_tensor("ys", [rbs[-1]], F16, kind="ExternalOutput")

    with tile.TileContext(nc) as tc, ExitStack() as ctx, \
            nc.allow_low_precision(reason="fp16/fp8 kernel, tol 2e-2"):
        spool = ctx.enter_context(tc.tile_pool(name="s", bufs=5))
        wpool = ctx.enter_context(tc.tile_pool(name="w", bufs=2))
        zpool = ctx.enter_context(tc.tile_pool(name="z", bufs=2))
        vpool = ctx.enter_context(tc.tile_pool(name="v", bufs=2))
        gpool = ctx.enter_context(tc.tile_pool(name="g", bufs=2))

        def emit_tail(fi, rb, zc, w2v, b2v, gp):
            # zd reuses the "za" tag: za(t) is dead once zb(t) exists.
            zd = zpool.tile([128, H * fi], F16, tag="za")
            nc.vector.tensor_tensor(zd[:], zc[:], w2v, op=OP.mult)
            u1 = vpool.tile([128, 4 * fi], F16, tag="u1")
            nc.vector.tensor_tensor(
                u1[:], zd[:, 0:4 * fi], zd[:, 4 * fi:8 * fi], op=OP.add
            )
            u2 = vpool.tile([128, 2 * fi], F16, tag="u2")
            nc.vector.tensor_tensor(
                u2[:], u1[:, 0:2 * fi], u1[:, 2 * fi:4 * fi], op=OP.add
            )
            if gp:
                # Offload the last two tree levels to the idle GPSIMD engine
                # for the big mid tiles: ~0.62us/tile off DVE for ~2.1us/tile
                # of GPSIMD (which has nothing downstream waiting on it - the
                # y-write rides the same in-order gpsimd queue).
                yt = gpool.tile([128, fi], F16, tag="ytg")
                nc.gpsimd.tensor_tensor(
                    yt[:], u2[:, 0:fi], u2[:, fi:2 * fi], op=OP.add
                )
                yo = gpool.tile([128, fi], F16, tag="yog")
                nc.gpsimd.tensor_tensor(yo[:], yt[:], b2v, op=OP.add)
            else:
                yt = vpool.tile([128, fi], F16, tag="yt")
                nc.vector.tensor_tensor(
                    yt[:], u2[:, 0:fi], u2[:, fi:2 * fi], op=OP.add
                )
                yo = vpool.tile([128, fi], F16, tag="yo")
                nc.vector.tensor_tensor(yo[:], yt[:], b2v, op=OP.add)
            # y-writes ride the GPSIMD software-DGE queue: keeps them off the
            # slab HW DGE queue and the ACT queue while those are busy. The
            # last two switch to the scalar HW queue (idle after the final
            # ACTIVATE) so the drain doesn't wait on SWDGE completion lag.
            # (Sync-queue routing for the tail was measured worse - it still
            # holds the slab stream's completion sems.)
            eng = nc.scalar if rb >= rbs[nt - 2] else nc.gpsimd
            eng.dma_start(
                ys.ap()[rb:rb + 128 * fi].rearrange("(p f) -> p f", p=128), yo[:]
            )

        def emit_dma(ti, engine):
            fi = fis[ti]
            S = spool.tile([128, slab_b(ti) * fi], U8, tag="slab")
            src = slab.ap()[offs[ti]:offs[ti + 1]].rearrange("(p k) -> p k", p=128)
            engine.dma_start(S[:], src)
            return S

        def emit_up(S, ti):
            fi = fis[ti]
            w1 = wpool.tile([128, H * fi], F16, tag="w1")
            nc.scalar.copy(w1[:], S[:, 0:8 * fi].bitcast(F8))
            return w1

        prev = None
        # Slab DMA rides two tiles ahead, upconvert one: up(t+1) sits before
        # relu(t) on the in-order ACT queue, and its slab was issued a full
        # period earlier so it never waits on a fresh transfer.
        # (A three-queue prelude - slab1 on gpsimd SWDGE, slab2 on scalar -
        # was tried to parallelize the ~3us first-transfer DGE startup
        # latency: measured 7us WORSE; SWDGE descriptor generation is too
        # slow for MB-scale transfers.)
        slabs = [emit_dma(0, nc.sync), emit_dma(1, nc.scalar)]
        ups = {t: emit_up(slabs[t], t) for t in (0,) if FP8T[t]}
        for ti, fi in enumerate(fis):
            S = slabs[ti]
            if ti + 2 < nt:
                slabs.append(emit_dma(ti + 2, nc.sync))
            if ti + 1 < nt and FP8T[ti + 1]:
                ups[ti + 1] = emit_up(slabs[ti + 1], ti + 1)

            if FP8T[ti]:
                w1v = ups[ti][:].rearrange("p (j f) -> p j f", j=H)
                o = 8 * fi
            else:
                w1v = S[:, 0:16 * fi].bitcast(F16).rearrange("p (j f) -> p j f", j=H)
                o = 16 * fi
            b1v = S[:, o:o + 16 * fi].bitcast(F16)
            w2v = S[:, o + 16 * fi:o + 32 * fi].bitcast(F16)
            xv = S[:, o + 32 * fi:o + 34 * fi].bitcast(F16)
            b2v = S[:, o + 34 * fi:o + 36 * fi].bitcast(F16)
            xb = xv.rearrange("p (o f) -> p o f", o=1).broadcast_to([128, H, fi])

            za = zpool.tile([128, H * fi], F16, tag="za")
            zb = zpool.tile([128, H * fi], F16, tag="zb")
            zc = zpool.tile([128, H * fi], F16, tag="zc")

            nc.vector.tensor_tensor(
                za[:].rearrange("p (j f) -> p j f", j=H), xb, w1v, op=OP.mult
            )
            nc.vector.tensor_tensor(zb[:], za[:], b1v, op=OP.add)
            if FP8T[ti]:
                nc.scalar.activation(zc[:], zb[:], AF.Relu)
            else:
                nc.vector.tensor_scalar_max(zc[:], zb[:], 0.0)

            if prev is not None:
                emit_tail(*prev)
            prev = (fi, rbs[ti], zc, w2v, b2v, fi == 480)
        emit_tail(*prev)

    nc.compile()
    return nc


# ---------------- entry point ----------------

_CACHE = {}


def _get_nc():
    if "nc" not in _CACHE:
        _CACHE["nc"] = build_nc(FIS)
    return _CACHE["nc"]


def _pack_core(w1u8, w1f16, b1, w2, xs, b2):
    """Build the interleaved j-major mixed-dtype slab for one core.

    w1u8 [R_PAD, 8] uint8 (e3m4 bytes), w1f16/b1/w2 [R_PAD, 8] fp16,
    xs/b2 [R_PAD] fp16. Tile t (fi nets/partition): net = rb + p*fi + f.
    """
    parts = []
    rb = 0
    for ti, fi in enumerate(FIS):
        nrows = 128 * fi
        jmaj = lambda a: np.ascontiguousarray(
            a[rb:rb + nrows].reshape(128, fi, H).transpose(0, 2, 1)
        ).reshape(128, H * fi)
        w1sec = jmaj(w1u8) if FP8T[ti] else jmaj(w1f16).view(np.uint8)
        t = np.concatenate(
            [
                w1sec,
                jmaj(b1).view(np.uint8),
                jmaj(w2).view(np.uint8),
                xs[rb:rb + nrows].reshape(128, fi).view(np.uint8),
                b2[rb:rb + nrows].reshape(128, fi).view(np.uint8),
            ],
            axis=1,
        )
        parts.append(t.reshape(-1))
        rb += nrows
    return np.concatenate(parts)


def _pad2(a, dt):
    out = np.zeros((R_PAD, H), dt)
    out[:R] = a
    return out


def _pad1(a):
    out = np.zeros(R_PAD, np.float16)
    out[:R] = a
    return out


def _make_in_maps(x, W1, b1, W2, b2):
    x = np.asarray(x, np.float16)
    W1f16 = np.asarray(W1, np.float16)
    W1u8 = np.asarray(W1, np.float32).astype(E3M4).view(np.uint8)
    b1 = np.asarray(b1, np.float16)
    W2 = np.asarray(W2, np.float16)
    b2 = np.asarray(b2, np.float16)
    in_maps = []
    for c in range(N_CORES):
        sl = slice(c * R, (c + 1) * R)
        in_maps.append({
            "slab": _pack_core(
                _pad2(W1u8[sl], np.uint8), _pad2(W1f16[sl], np.float16),
                _pad2(b1[sl], np.float16), _pad2(W2[sl], np.float16),
                _pad1(x[sl, 0]), _pad1(b2[sl, 0]),
            ),
        })
    return in_maps


def _run(x, W1, b1, W2, b2, **kw):
    nc = _get_nc()
    res = run_bass_kernel_spmd(nc, _make_in_maps(x, W1, b1, W2, b2),
                               core_ids=list(range(N_CORES)), **kw)
    y = np.empty((N, 1), np.float32)
    for c in range(N_CORES):
        y[c * R:(c + 1) * R, 0] = res.results[c]["ys"].reshape(-1)[:R].astype(
            np.float32
        )
    return y, res


def kernel(x, W1, b1, W2, b2):
    y, _ = _run(x, W1, b1, W2, b2)
    return y


# revision 35
# speedup vs baseline: 1.1028x; 1.1028x over previous
"""Trainium2 Bass kernel for 4M per-element tiny MLPs (1->8->1, H=8).

    y[i] = W2[i] @ relu(W1[i] * x[i] + b1[i]) + b2[i]

Memory-bound; data-parallel over 8 NeuronCores (500k nets/core), no
communication.

Design (evolution of the fp16 j-major slab baseline, 104.8 us):
  * Mid tiles store W1 as fp8 e3m4 (TRN float8e3: 4 mantissa bits; W1 in
    (-1,1) so no overflow; end-to-end rel_l2 7.5e-3 vs the 2e-2 gate).
    Slab 52 -> 44 B/net; the fp8->fp16 upconvert runs on the ACT engine,
    which has slack (relu+upconvert ~6.8 us < DVE ~8.4 us per 480-tile).
    DVE op mix is untouched (its operands stay fp16 at 2x_1p).
  * Edge tiles (first 2, last 2) keep W1 in fp16 and run relu on DVE
    (tensor_scalar_max @ 4x): their za/zb/relu/tail chains touch no other
    engine, so pipeline fill and drain carry no cross-engine sem
    latency. Costs ~1.5 us of DVE across the 739 edge nets/partition.
  * ALL slab DMAs ride the sync queue, issued 2 tiles ahead (except
    slab 1, issued on the scalar queue before any ACTIVATE so the first
    two transfers overlap). v3's lesson: a DMA_DIRECT2D queued on the
    scalar engine right before its consuming ACTIVATE head-of-line
    blocks the ACT queue for the whole transfer. With fp8 bytes + big
    tiles one HW DGE queue sustains ~430 GB/s and fits in the period.
  * ACT queue order per tile: upconv(t+1) then relu(t) - upconv rides
    one tile ahead so relu's zb-wait never delays it.
  * y-writes go via the GPSIMD software-DGE queue (keeps them off both
    the HW DGE queue and the ACT queue), except the last two tiles',
    which ride the by-then-idle scalar HW queue so the final drain
    barrier does not wait on SWDGE completion lag. Output is fp16.
  * Slab = one uint8 DMA per tile; typed views via bitcast:
    fp8 tile: [W1 f8 j-major 8fi | b1 f16 j-major 16fi | W2 f16 j-major
    16fi | x f16 2fi | b2 f16 2fi] B/partition (44fi); fp16 tile: same
    with W1 f16 16fi first (52fi).

Measured (run-to-run spread +-3-6us, HBM shared with co-tenants):
v1 all-fp16 104.8us (DMA-period-bound: 81.7us DMA active > 74.9 DVE);
fp8+ACT-upconvert with naive queueing 117-119us (ACT queue overloaded by
DMA_DIRECT2D + upconverts, and up(t+1) queued behind relu(t) serialized
the pipe); single-sync-queue DMA + 2-ahead prefetch + 1-ahead upconvert
101.4us; +fp16/DVE-relu edge tiles 101.2us; +last-2 y-outs on the idle
scalar HW queue 101.06us (best; DVE-bound: ~75.4us busy, DMA active 61,
ACT 50; remaining span = ~5.5us NEFF preamble + ~3us first-transfer DGE
startup latency + ~9us ramp fill + ~8.5us drain barrier). Dead ends
measured: upconvert hoisted 2 ahead with DMA 3 ahead (+4us); y-outs via
sync queue (+4us; sync still holds slab completion sems); finer 13-tile
taper (+4.5us: per-tile op+sem overhead beats ramp savings); 3-queue
prelude with slab1 on gpsimd SWDGE (+7.8us: SWDGE descriptor gen too
slow for MB transfers); PE-engine tail reduction (ones-matmul
j-on-partitions, verified exact in mini_pe.py) - killed by "DMA cannot
read PSUM": the PSUM->SBUF copy at 1 elem/net on 16 partitions costs
more than the DVE tree it removes. Structural wall: DVE 16 cyc/net
(tensor_tensor 2x floor; custom DVE ops are 1x, GPSIMD ~4x slower +
steals the DVE SBUF port).
"""

import numpy as np
import ml_dtypes
from contextlib import ExitStack

import concourse.bacc as bacc
import concourse.mybir as mybir
import concourse.tile as tile
from concourse.bass_utils import run_bass_kernel_spmd

F16 = mybir.dt.float16
F8 = mybir.dt.float8e3
U8 = mybir.dt.uint8
AF = mybir.ActivationFunctionType
OP = mybir.AluOpType
E3M4 = ml_dtypes.float8_e3m4

N = 4_000_000
H = 8
N_CORES = 8
R = N // N_CORES            # 500,000 nets per core
FP = 3907                   # nets per partition (padded): 128*3907 = 500,096
R_PAD = 128 * FP
FIS = [64, 224, 288] + [480] * 6 + [252, 199]
FP8T = [False, False, True, True, True, True, True, True, True, False, False]


def slab_b(ti):
    return 44 if FP8T[ti] else 52


def build_nc(fis):
    nt = len(fis)
    tile_bytes = [128 * fis[t] * slab_b(t) for t in range(nt)]
    offs = [0]
    for tb in tile_bytes:
        offs.append(offs[-1] + tb)
    rbs = [0]
    for fi in fis:
        rbs.append(rbs[-1] + 128 * fi)

    nc = bacc.Bacc("TRN2", target_bir_lowering=False, debug=False)

    slab = nc.dram_tensor("slab", [offs[-1]], U8, kind="ExternalInput")
    ys = nc.dram_tensor("ys", [rbs[-1]], F16, kind="ExternalOutput")

    with tile.TileContext(nc) as tc, ExitStack() as ctx, \
            nc.allow_low_precision(reason="fp16/fp8 kernel, tol 2e-2"):
        spool = ctx.enter_context(tc.tile_pool(name="s", bufs=5))
        wpool = ctx.enter_context(tc.tile_pool(name="w", bufs=2))
        zpool = ctx.enter_context(tc.tile_pool(name="z", bufs=2))
        vpool = ctx.enter_context(tc.tile_pool(name="v", bufs=2))

        def emit_tail(fi, rb, zc, w2v, b2v):
            # zd reuses the "za" tag: za(t) is dead once zb(t) exists.
            zd = zpool.tile([128, H * fi], F16, tag="za")
            nc.vector.tensor_tensor(zd[:], zc[:], w2v, op=OP.mult)
            u1 = vpool.tile([128, 4 * fi], F16, tag="u1")
            nc.vector.tensor_tensor(
                u1[:], zd[:, 0:4 * fi], zd[:, 4 * fi:8 * fi], op=OP.add
            )
            u2 = vpool.tile([128, 2 * fi], F16, tag="u2")
            nc.vector.tensor_tensor(
                u2[:], u1[:, 0:2 * fi], u1[:, 2 * fi:4 * fi], op=OP.add
            )
            yt = vpool.tile([128, fi], F16, tag="yt")
            nc.vector.tensor_tensor(yt[:], u2[:, 0:fi], u2[:, fi:2 * fi], op=OP.add)
            yo = vpool.tile([128, fi], F16, tag="yo")
            nc.vector.tensor_tensor(yo[:], yt[:], b2v, op=OP.add)
            # y-writes ride the GPSIMD software-DGE queue: keeps them off the
            # slab HW DGE queue and the ACT queue while those are busy. The
            # last two switch to the scalar HW queue (idle after the final
            # ACTIVATE) so the drain doesn't wait on SWDGE completion lag.
            # (Sync-queue routing for the tail was measured worse - it still
            # holds the slab stream's completion sems.)
            eng = nc.scalar if rb >= rbs[nt - 2] else nc.gpsimd
            eng.dma_start(
                ys.ap()[rb:rb + 128 * fi].rearrange("(p f) -> p f", p=128), yo[:]
            )

        def emit_dma(ti, engine):
            fi = fis[ti]
            S = spool.tile([128, slab_b(ti) * fi], U8, tag="slab")
            src = slab.ap()[offs[ti]:offs[ti + 1]].rearrange("(p k) -> p k", p=128)
            engine.dma_start(S[:], src)
            return S

        def emit_up(S, ti):
            fi = fis[ti]
            w1 = wpool.tile([128, H * fi], F16, tag="w1")
            nc.scalar.copy(w1[:], S[:, 0:8 * fi].bitcast(F8))
            return w1

        prev = None
        # Slab DMA rides two tiles ahead, upconvert one: up(t+1) sits before
        # relu(t) on the in-order ACT queue, and its slab was issued a full
        # period earlier so it never waits on a fresh transfer.
        # (A three-queue prelude - slab1 on gpsimd SWDGE, slab2 on scalar -
        # was tried to parallelize the ~3us first-transfer DGE startup
        # latency: measured 7us WORSE; SWDGE descriptor generation is too
        # slow for MB-scale transfers.)
        slabs = [emit_dma(0, nc.sync), emit_dma(1, nc.scalar)]
        ups = {t: emit_up(slabs[t], t) for t in (0,) if FP8T[t]}
        for ti, fi in enumerate(fis):
            S = slabs[ti]
            if ti + 2 < nt:
                slabs.append(emit_dma(ti + 2, nc.sync))
            if ti + 1 < nt and FP8T[ti + 1]:
                ups[ti + 1] = emit_up(slabs[ti + 1], ti + 1)

            if FP8T[ti]:
                w1v = ups[ti][:].rearrange("p (j f) -> p j f", j=H)
                o = 8 * fi
            else:
                w1v = S[:, 0:16 * fi].bitcast(F16).rearrange("p (j f) -> p j f", j=H)
                o = 16 * fi
            b1v = S[:, o:o + 16 * fi].bitcast(F16)
            w2v = S[:, o + 16 * fi:o + 32 * fi].bitcast(F16)
            xv = S[:, o + 32 * fi:o + 34 * fi].bitcast(F16)
            b2v = S[:, o + 34 * fi:o + 36 * fi].bitcast(F16)
            xb = xv.rearrange("p (o f) -> p o f", o=1).broadcast_to([128, H, fi])

            za = zpool.tile([128, H * fi], F16, tag="za")
            zb = zpool.tile([128, H * fi], F16, tag="zb")
            zc = zpool.tile([128, H * fi], F16, tag="zc")

            nc.vector.tensor_tensor(
                za[:].rearrange("p (j f) -> p j f", j=H), xb, w1v, op=OP.mult
            )
            nc.vector.tensor_tensor(zb[:], za[:], b1v, op=OP.add)
            if FP8T[ti]:
                nc.scalar.activation(zc[:], zb[:], AF.Relu)
            else:
                nc.vector.tensor_scalar_max(zc[:], zb[:], 0.0)

            if prev is not None:
                emit_tail(*prev)
            prev = (fi, rbs[ti], zc, w2v, b2v)
        emit_tail(*prev)

    nc.compile()
    return nc


# ---------------- entry point ----------------

_CACHE = {}


def _get_nc():
    if "nc" not in _CACHE:
        _CACHE["nc"] = build_nc(FIS)
    return _CACHE["nc"]


def _pack_core(w1u8, w1f16, b1, w2, xs, b2):
    """Build the interleaved j-major mixed-dtype slab for one core.

    w1u8 [R_PAD, 8] uint8 (e3m4 bytes), w1f16/b1/w2 [R_PAD, 8] fp16,
    xs/b2 [R_PAD] fp16. Tile t (fi nets/partition): net = rb + p*fi + f.
    """
    parts = []
    rb = 0
    for ti, fi in enumerate(FIS):
        nrows = 128 * fi
        jmaj = lambda a: np.ascontiguousarray(
            a[rb:rb + nrows].reshape(128, fi, H).transpose(0, 2, 1)
        ).reshape(128, H * fi)
        w1sec = jmaj(w1u8) if FP8T[ti] else jmaj(w1f16).view(np.uint8)
        t = np.concatenate(
            [
                w1sec,
                jmaj(b1).view(np.uint8),
                jmaj(w2).view(np.uint8),
                xs[rb:rb + nrows].reshape(128, fi).view(np.uint8),
                b2[rb:rb + nrows].reshape(128, fi).view(np.uint8),
            ],
            axis=1,
        )
        parts.append(t.reshape(-1))
        rb += nrows
    return np.concatenate(parts)


def _pad2(a, dt):
    out = np.zeros((R_PAD, H), dt)
    out[:R] = a
    return out


def _pad1(a):
    out = np.zeros(R_PAD, np.float16)
    out[:R] = a
    return out


def _make_in_maps(x, W1, b1, W2, b2):
    x = np.asarray(x, np.float16)
    W1f16 = np.asarray(W1, np.float16)
    W1u8 = np.asarray(W1, np.float32).astype(E3M4).view(np.uint8)
    b1 = np.asarray(b1, np.float16)
    W2 = np.asarray(W2, np.float16)
    b2 = np.asarray(b2, np.float16)
    in_maps = []
    for c in range(N_CORES):
        sl = slice(c * R, (c + 1) * R)
        in_maps.append({
            "slab": _pack_core(
                _pad2(W1u8[sl], np.uint8), _pad2(W1f16[sl], np.float16),
                _pad2(b1[sl], np.float16), _pad2(W2[sl], np.float16),
                _pad1(x[sl, 0]), _pad1(b2[sl, 0]),
            ),
        })
    return in_maps


def _run(x, W1, b1, W2, b2, **kw):
    nc = _get_nc()
    res = run_bass_kernel_spmd(nc, _make_in_maps(x, W1, b1, W2, b2),
                               core_ids=list(range(N_CORES)), **kw)
    y = np.empty((N, 1), np.float32)
    for c in range(N_CORES):
        y[c * R:(c + 1) * R, 0] = res.results[c]["ys"].reshape(-1)[:R].astype(
            np.float32
        )
    return y, res


def kernel(x, W1, b1, W2, b2):
    y, _ = _run(x, W1, b1, W2, b2)
    return y


# revision 36
# speedup vs baseline: 1.1143x; 1.0104x over previous
"""Trainium2 Bass kernel for 4M per-element tiny MLPs (1->8->1, H=8).

    y[i] = W2[i] @ relu(W1[i] * x[i] + b1[i]) + b2[i]

Memory-bound; data-parallel over 8 NeuronCores (500k nets/core), no
communication.

Design (evolution of the fp16 j-major slab baseline, 104.8 us):
  * Mid tiles store W1 as fp8 e3m4 (TRN float8e3: 4 mantissa bits; W1 in
    (-1,1) so no overflow; end-to-end rel_l2 7.5e-3 vs the 2e-2 gate).
    Slab 52 -> 44 B/net; the fp8->fp16 upconvert runs on the ACT engine,
    which has slack (relu+upconvert ~6.8 us < DVE ~8.4 us per 480-tile).
    DVE op mix is untouched (its operands stay fp16 at 2x_1p).
  * Edge tiles (first 2, last 2) keep W1 in fp16 and run relu on DVE
    (tensor_scalar_max @ 4x): their za/zb/relu/tail chains touch no other
    engine, so pipeline fill and drain carry no cross-engine sem
    latency. Costs ~1.5 us of DVE across the 739 edge nets/partition.
  * ALL slab DMAs ride the sync queue, issued 2 tiles ahead (except
    slab 1, issued on the scalar queue before any ACTIVATE so the first
    two transfers overlap). v3's lesson: a DMA_DIRECT2D queued on the
    scalar engine right before its consuming ACTIVATE head-of-line
    blocks the ACT queue for the whole transfer. With fp8 bytes + big
    tiles one HW DGE queue sustains ~430 GB/s and fits in the period.
  * ACT queue order per tile: upconv(t+1) then relu(t) - upconv rides
    one tile ahead so relu's zb-wait never delays it.
  * y-writes go via the GPSIMD software-DGE queue (keeps them off both
    the HW DGE queue and the ACT queue), except the last two tiles',
    which ride the by-then-idle scalar HW queue so the final drain
    barrier does not wait on SWDGE completion lag. Output is fp16.
  * Slab = one uint8 DMA per tile; typed views via bitcast:
    fp8 tile: [W1 f8 j-major 8fi | b1 f16 j-major 16fi | W2 f16 j-major
    16fi | x f16 2fi | b2 f16 2fi] B/partition (44fi); fp16 tile: same
    with W1 f16 16fi first (52fi).

Measured (run-to-run spread +-3-6us, HBM shared with co-tenants):
v1 all-fp16 104.8us (DMA-period-bound: 81.7us DMA active > 74.9 DVE);
fp8+ACT-upconvert with naive queueing 117-119us (ACT queue overloaded by
DMA_DIRECT2D + upconverts, and up(t+1) queued behind relu(t) serialized
the pipe); single-sync-queue DMA + 2-ahead prefetch + 1-ahead upconvert
101.4us; +fp16/DVE-relu edge tiles 101.2us; +last-2 y-outs on the idle
scalar HW queue = this config: 100.75/100.91/101.06/101.23/101.75/102.0/
103.3 over seven runs, best 100753 ns (DVE-bound: ~75.4us busy, DMA
active 61, ACT 50; remaining span = ~5.5us NEFF preamble + ~3us
first-transfer DGE startup latency + ~9us HBM-limited ramp fill +
~8.5us TileContext drain barrier). Dead ends
measured: upconvert hoisted 2 ahead with DMA 3 ahead (+4us); y-outs via
sync queue (+4us; sync still holds slab completion sems); finer 13-tile
taper (+4.5us: per-tile op+sem overhead beats ramp savings); 3-queue
prelude with slab1 on gpsimd SWDGE (+7.8us: SWDGE descriptor gen too
slow for MB transfers); PE-engine tail reduction (ones-matmul
j-on-partitions, verified exact in mini_pe.py) - killed by "DMA cannot
read PSUM": the PSUM->SBUF copy at 1 elem/net on 16 partitions costs
more than the DVE tree it removes; GPSIMD yt/yo tree offload for the
480-tiles (112.5us: SBUF port contention costs the pipeline ~2us/tile
for 0.62us/tile of DVE savings); tensor_tensor_reduce fuses mult+reduce
but its accum_out is [P,1] (whole-free-dim) - no per-net grouping.
Structural wall: DVE 16 cyc/net (tensor_tensor 2x floor; custom DVE ops
are 1x; the only unexplored lever is authoring a 2x-mode DVE uop).
"""

import numpy as np
import ml_dtypes
from contextlib import ExitStack

import concourse.bacc as bacc
import concourse.mybir as mybir
import concourse.tile as tile
from concourse.bass_utils import run_bass_kernel_spmd

F16 = mybir.dt.float16
F8 = mybir.dt.float8e3
U8 = mybir.dt.uint8
AF = mybir.ActivationFunctionType
OP = mybir.AluOpType
E3M4 = ml_dtypes.float8_e3m4

N = 4_000_000
H = 8
N_CORES = 8
R = N // N_CORES            # 500,000 nets per core
FP = 3907                   # nets per partition (padded): 128*3907 = 500,096
R_PAD = 128 * FP
FIS = [64, 224, 288] + [480] * 6 + [252, 199]
FP8T = [False, False, True, True, True, True, True, True, True, False, False]


def slab_b(ti):
    return 44 if FP8T[ti] else 52


def build_nc(fis):
    nt = len(fis)
    tile_bytes = [128 * fis[t] * slab_b(t) for t in range(nt)]
    offs = [0]
    for tb in tile_bytes:
        offs.append(offs[-1] + tb)
    rbs = [0]
    for fi in fis:
        rbs.append(rbs[-1] + 128 * fi)

    nc = bacc.Bacc("TRN2", target_bir_lowering=False, debug=False)

    slab = nc.dram_tensor("slab", [offs[-1]], U8, kind="ExternalInput")
    ys = nc.dram_tensor("ys", [rbs[-1]], F16, kind="ExternalOutput")

    with tile.TileContext(nc) as tc, ExitStack() as ctx, \
            nc.allow_low_precision(reason="fp16/fp8 kernel, tol 2e-2"):
        spool = ctx.enter_context(tc.tile_pool(name="s", bufs=5))
        wpool = ctx.enter_context(tc.tile_pool(name="w", bufs=2))
        zpool = ctx.enter_context(tc.tile_pool(name="z", bufs=2))
        vpool = ctx.enter_context(tc.tile_pool(name="v", bufs=2))

        def emit_tail(fi, rb, zc, w2v, b2v):
            # zd reuses the "za" tag: za(t) is dead once zb(t) exists.
            zd = zpool.tile([128, H * fi], F16, tag="za")
            nc.vector.tensor_tensor(zd[:], zc[:], w2v, op=OP.mult)
            u1 = vpool.tile([128, 4 * fi], F16, tag="u1")
            nc.vector.tensor_tensor(
                u1[:], zd[:, 0:4 * fi], zd[:, 4 * fi:8 * fi], op=OP.add
            )
            u2 = vpool.tile([128, 2 * fi], F16, tag="u2")
            nc.vector.tensor_tensor(
                u2[:], u1[:, 0:2 * fi], u1[:, 2 * fi:4 * fi], op=OP.add
            )
            yt = vpool.tile([128, fi], F16, tag="yt")
            nc.vector.tensor_tensor(yt[:], u2[:, 0:fi], u2[:, fi:2 * fi], op=OP.add)
            yo = vpool.tile([128, fi], F16, tag="yo")
            nc.vector.tensor_tensor(yo[:], yt[:], b2v, op=OP.add)
            # y-writes ride the GPSIMD software-DGE queue: keeps them off the
            # slab HW DGE queue and the ACT queue while those are busy. The
            # last two switch to the scalar HW queue (idle after the final
            # ACTIVATE) so the drain doesn't wait on SWDGE completion lag.
            # (Sync-queue routing for the tail was measured worse - it still
            # holds the slab stream's completion sems.)
            eng = nc.scalar if rb >= rbs[nt - 2] else nc.gpsimd
            eng.dma_start(
                ys.ap()[rb:rb + 128 * fi].rearrange("(p f) -> p f", p=128), yo[:]
            )

        def emit_dma(ti, engine):
            fi = fis[ti]
            S = spool.tile([128, slab_b(ti) * fi], U8, tag="slab")
            src = slab.ap()[offs[ti]:offs[ti + 1]].rearrange("(p k) -> p k", p=128)
            engine.dma_start(S[:], src)
            return S

        def emit_up(S, ti):
            fi = fis[ti]
            w1 = wpool.tile([128, H * fi], F16, tag="w1")
            nc.scalar.copy(w1[:], S[:, 0:8 * fi].bitcast(F8))
            return w1

        prev = None
        # Slab DMA rides two tiles ahead, upconvert one: up(t+1) sits before
        # relu(t) on the in-order ACT queue, and its slab was issued a full
        # period earlier so it never waits on a fresh transfer.
        # (A three-queue prelude - slab1 on gpsimd SWDGE, slab2 on scalar -
        # was tried to parallelize the ~3us first-transfer DGE startup
        # latency: measured 7us WORSE; SWDGE descriptor generation is too
        # slow for MB-scale transfers.)
        slabs = [emit_dma(0, nc.sync), emit_dma(1, nc.scalar)]
        ups = {t: emit_up(slabs[t], t) for t in (0,) if FP8T[t]}
        for ti, fi in enumerate(fis):
            S = slabs[ti]
            if ti + 2 < nt:
                slabs.append(emit_dma(ti + 2, nc.sync))
            if ti + 1 < nt and FP8T[ti + 1]:
                ups[ti + 1] = emit_up(slabs[ti + 1], ti + 1)

            if FP8T[ti]:
                w1v = ups[ti][:].rearrange("p (j f) -> p j f", j=H)
                o = 8 * fi
            else:
                w1v = S[:, 0:16 * fi].bitcast(F16).rearrange("p (j f) -> p j f", j=H)
                o = 16 * fi
            b1v = S[:, o:o + 16 * fi].bitcast(F16)
            w2v = S[:, o + 16 * fi:o + 32 * fi].bitcast(F16)
            xv = S[:, o + 32 * fi:o + 34 * fi].bitcast(F16)
            b2v = S[:, o + 34 * fi:o + 36 * fi].bitcast(F16)
            xb = xv.rearrange("p (o f) -> p o f", o=1).broadcast_to([128, H, fi])

            za = zpool.tile([128, H * fi], F16, tag="za")
            zb = zpool.tile([128, H * fi], F16, tag="zb")
            zc = zpool.tile([128, H * fi], F16, tag="zc")

            nc.vector.tensor_tensor(
                za[:].rearrange("p (j f) -> p j f", j=H), xb, w1v, op=OP.mult
            )
            nc.vector.tensor_tensor(zb[:], za[:], b1v, op=OP.add)
            if FP8T[ti]:
                nc.scalar.activation(zc[:], zb[:], AF.Relu)
            else:
                nc.vector.tensor_scalar_max(zc[:], zb[:], 0.0)

            if prev is not None:
                emit_tail(*prev)
            prev = (fi, rbs[ti], zc, w2v, b2v)
        emit_tail(*prev)

    nc.compile()
    return nc


# ---------------- entry point ----------------

_CACHE = {}


def _get_nc():
    if "nc" not in _CACHE:
        _CACHE["nc"] = build_nc(FIS)
    return _CACHE["nc"]


def _pack_core(w1u8, w1f16, b1, w2, xs, b2):
    """Build the interleaved j-major mixed-dtype slab for one core.

    w1u8 [R_PAD, 8] uint8 (e3m4 bytes), w1f16/b1/w2 [R_PAD, 8] fp16,
    xs/b2 [R_PAD] fp16. Tile t (fi nets/partition): net = rb + p*fi + f.
    """
    parts = []
    rb = 0
    for ti, fi in enumerate(FIS):
        nrows = 128 * fi
        jmaj = lambda a: np.ascontiguousarray(
            a[rb:rb + nrows].reshape(128, fi, H).transpose(0, 2, 1)
        ).reshape(128, H * fi)
        w1sec = jmaj(w1u8) if FP8T[ti] else jmaj(w1f16).view(np.uint8)
        t = np.concatenate(
            [
                w1sec,
                jmaj(b1).view(np.uint8),
                jmaj(w2).view(np.uint8),
                xs[rb:rb + nrows].reshape(128, fi).view(np.uint8),
                b2[rb:rb + nrows].reshape(128, fi).view(np.uint8),
            ],
            axis=1,
        )
        parts.append(t.reshape(-1))
        rb += nrows
    return np.concatenate(parts)


def _pad2(a, dt):
    out = np.zeros((R_PAD, H), dt)
    out[:R] = a
    return out


def _pad1(a):
    out = np.zeros(R_PAD, np.float16)
    out[:R] = a
    return out


def _make_in_maps(x, W1, b1, W2, b2):
    x = np.asarray(x, np.float16)
    W1f16 = np.asarray(W1, np.float16)
    W1u8 = np.asarray(W1, np.float32).astype(E3M4).view(np.uint8)
    b1 = np.asarray(b1, np.float16)
    W2 = np.asarray(W2, np.float16)
    b2 = np.asarray(b2, np.float16)
    in_maps = []
    for c in range(N_CORES):
        sl = slice(c * R, (c + 1) * R)
        in_maps.append({
            "slab": _pack_core(
                _pad2(W1u8[sl], np.uint8), _pad2(W1f16[sl], np.float16),
                _pad2(b1[sl], np.float16), _pad2(W2[sl], np.float16),
                _pad1(x[sl, 0]), _pad1(b2[sl, 0]),
            ),
        })
    return in_maps


def _run(x, W1, b1, W2, b2, **kw):
    nc = _get_nc()
    res = run_bass_kernel_spmd(nc, _make_in_maps(x, W1, b1, W2, b2),
                               core_ids=list(range(N_CORES)), **kw)
    y = np.empty((N, 1), np.float32)
    for c in range(N_CORES):
        y[c * R:(c + 1) * R, 0] = res.results[c]["ys"].reshape(-1)[:R].astype(
            np.float32
        )
    return y, res


def kernel(x, W1, b1, W2, b2):
    y, _ = _run(x, W1, b1, W2, b2)
    return y
